# revision 10
# baseline (speedup 1.0000x reference)
"""Trainium2 Bass kernel for nn_MessagePassingConv (GNN message passing) — v3.

Strategy (8 NeuronCores, SPMD):
- Nodes sharded by row range: core c owns target nodes [c*62500, (c+1)*62500).
- Edges sharded by TARGET core; x replicated (f16 copy) so source gathers are
  local. Only a [128,2] BN-stat AllReduce crosses cores.
- W=128 target windows; groups of GP=4 positions processed together.
  Per-core processing order sorts positions by edge count so the shared SPMD
  block counts B[i] track quantiles, not maxima.
- Per group: ONE batched indirect-DMA gather (all blocks, pads skipped via
  bounds_check), ONE broadcast is_equal builds all one-hot blocks, scatter
  matmuls (f16) accumulate S^T slices in a [128,1024] PSUM tile;
  aggre^T = wp^T Sp^T + wn^T Sn^T + I xt in PSUM; relu(aggre + b) lands in an
  SBUF-resident A [128, 62500] f16 with BN-sum accum; sq-sum via fused
  tensor_tensor_reduce on DVE.
- BN shift folded into GRU gate biases (3 tiny matmuls); BN scale folded into
  the GRU x-side kernel weights (one tensor_scalar) so pass 2 reads A raw.
- Pass 2 (GRU) in chunks of 1536 nodes: f16 matmuls; t2=hp+u folded into PSUM
  via identity matmul; sigmoid/tanh on ACT; dd/oo2/oo on DVE; output f16.
"""
import os
import sys
import types
import contextlib

import numpy as np

import concourse.bass as bass
import concourse.mybir as mybir
import concourse.tile as tile
from concourse.bass import IndirectOffsetOnAxis
from concourse.vector_clock import ScopedClock

F = 128
NCORE = 8
W = 512          # scatter window (targets per position)
GP = 1           # positions per group
BN_EPS = 1e-3
AF = mybir.ActivationFunctionType
ALU = mybir.AluOpType
f32 = mybir.dt.float32
f16 = mybir.dt.float16
i32 = mybir.dt.int32
BF = np.float16

# ---------------------------------------------------------------- patches

_MAXW = 1


def _patched_drain_and_barrier(self, tick_clock, wait_clock):
    nc = self.nc
    drain_inst = nc.sync.drain()
    wait_clock.add_sem_waits(
        drain_inst.ins, ScopedClock({None: tick_clock.global_clock})
    )
    si = drain_inst.ins.sync_info
    waits = list(si.on_wait) if si and si.on_wait else []
    if len(waits) > _MAXW:
        drain_inst.ins.sync_info = mybir.SyncInfo(
            on_wait=waits[:_MAXW], on_update=list(si.on_update or []))
        for i in range(_MAXW, len(waits), _MAXW):
            extra = nc.sync.drain()
            extra.ins.sync_info = mybir.SyncInfo(
                on_wait=waits[i:i + _MAXW], on_update=[])
    nc.all_engine_barrier()
    assert self.sems is not None
    popped = nc._tile_sem_poison_stack.pop()
    assert popped is self._sem_poison
    nc.clear_and_free_semaphores(list(self.sems.allocated().values()))
    nc.all_engine_barrier()


tile.TileContext._drain_and_barrier = _patched_drain_and_barrier

_split_n = [0]


def _split_excess_waits(nc, cap=1):
    """Walrus codegen caps sync-waits per instruction; move excess onto
    preceding same-engine InstNoOps."""
    for f in nc.m.functions:
        for blk in f.blocks:
            insts = blk.instructions
            if not any(i.sync_info and i.sync_info.on_wait
                       and len(i.sync_info.on_wait) > cap for i in insts):
                continue
            new = []
            for inst in insts:
                si = inst.sync_info
                waits = list(si.on_wait) if si and si.on_wait else []
                if len(waits) > cap:
                    keep, excess = waits[:cap], waits[cap:]
                    for j in range(0, len(excess), cap):
                        _split_n[0] += 1
                        nop = mybir.InstNoOp(
                            name=f"waitsplit-{_split_n[0]}",
                            ins=[], outs=[], engine=inst.engine)
                        nop.sync_info = mybir.SyncInfo(
                            on_wait=excess[j:j + cap], on_update=[])
                        new.append(nop)
                    inst.sync_info = mybir.SyncInfo(
                        on_wait=keep, on_update=list(si.on_update or []))
                new.append(inst)
            blk.instructions = new


def _install_ntff_hook():
    """Provide antenv.axon_hooks (missing in image) so trace=True works."""
    import ctypes
    if "antenv.axon_hooks" in sys.modules:
        return
    try:
        lib = ctypes.CDLL("/opt/axon/libaxon_pjrt.so")
    except OSError:
        return
    if not hasattr(lib, "axon_start_nrt_profile"):
        return
    lib.axon_start_nrt_profile.argtypes = [
        ctypes.POINTER(ctypes.c_int64), ctypes.c_size_t]
    lib.axon_start_nrt_profile.restype = ctypes.c_int64
    lib.axon_stop_nrt_profile.argtypes = [ctypes.c_char_p]
    lib.axon_stop_nrt_profile.restype = ctypes.c_int64

    @contextlib.contextmanager
    def _hook(output_dir, device_ids):
        import jax
        jax.devices()
        if device_ids:
            ids = (ctypes.c_int64 * len(device_ids))(*device_ids)
            rc = lib.axon_start_nrt_profile(ids, len(device_ids))
        else:
            rc = lib.axon_start_nrt_profile(None, 0)
        if rc != 0:
            raise RuntimeError(f"axon_start_nrt_profile rc={rc}")
        try:
            yield
        finally:
            n = lib.axon_stop_nrt_profile(str(output_dir).encode())
            print(f"profile: {n} file(s) written to {output_dir}")

    mod = types.ModuleType("antenv.axon_hooks")
    mod.get_axon_ntff_profile_hook = lambda: _hook
    mod.set_axon_ntff_profile_hook = lambda h: None
    import antenv
    sys.modules["antenv.axon_hooks"] = mod
    antenv.axon_hooks = mod
    from concourse import bass_utils
    bass_utils.upload_artifacts = lambda tmpdir: tmpdir


# ---------------------------------------------------------------- host prep

def _edge_layout(pairs, ncn, npos):
    """Per-core (src sorted by (bin, src), estart) and cnt [NCORE, npos]."""
    t = np.asarray(pairs[:, 0], dtype=np.int64)
    s = np.asarray(pairs[:, 1], dtype=np.int64)
    per_core = []
    cnt = np.zeros((NCORE, npos), dtype=np.int64)
    for c in range(NCORE):
        m = (t >= c * ncn) & (t < (c + 1) * ncn)
        tl = t[m] - c * ncn
        sc = s[m]
        binid = tl // W
        order = np.lexsort((sc, binid))      # bin-major, src-minor
        tl = tl[order]
        sc = sc[order]
        binid = binid[order]
        cnt[c] = np.bincount(binid, minlength=npos)
        estart = np.zeros(npos + 1, dtype=np.int64)
        np.cumsum(cnt[c], out=estart[1:])
        per_core.append((tl, sc, binid))
    return per_core, cnt


def _prepare(inputs):
    x = np.asarray(inputs["x"], dtype=np.float32)
    n_nodes = x.shape[0]
    ncn = n_nodes // NCORE
    npos = (ncn + W - 1) // W                     # 489
    wlast = ncn - (npos - 1) * W                  # 36

    pc_p, cnt_p = _edge_layout(inputs["pairs_prev"], ncn, npos)
    pc_n, cnt_n = _edge_layout(inputs["pairs_next"], ncn, npos)

    # per-core processing order: full bins sorted lexicographically by
    # (ceil_p/128, ceil_n/128, total) so per-direction block counts align
    # across cores at each rank; partial bin last
    orders = np.zeros((NCORE, npos), dtype=np.int64)
    for c in range(NCORE):
        bp = -(-cnt_p[c, :npos - 1] // 128)
        bn = -(-cnt_n[c, :npos - 1] // 128)
        key = ((bp * 64 + bn) * 100000
               + cnt_p[c, :npos - 1] + cnt_n[c, :npos - 1])
        orders[c, :npos - 1] = np.argsort(key, kind="stable")
        orders[c, npos - 1] = npos - 1

    # shared (SPMD) block counts per processing position, per direction
    def blocks(cnt):
        per_pos = np.take_along_axis(cnt, orders, axis=1)  # [NCORE, npos]
        return np.maximum(1, -(-per_pos // 128)).max(axis=0)  # [npos]

    Bd = [blocks(cnt_p), blocks(cnt_n)]           # [2][npos]

    # group structure: GP positions per group, block table layout
    # order: group -> position-in-group -> dir -> block
    ngroups = (npos + GP - 1) // GP
    first_col = np.zeros((2, npos), dtype=np.int64)
    blockmeta = []       # per group: list of (col, slice_off, start, stop)
    gbase = np.zeros(ngroups + 1, dtype=np.int64)
    col = 0
    for g in range(ngroups):
        metas = []
        for il, i in enumerate(range(g * GP, min((g + 1) * GP, npos))):
            for d in range(2):
                nb = int(Bd[d][i])
                first_col[d, i] = col
                for b in range(nb):
                    metas.append((col, d * (GP * W) + il * W,
                                  b == 0, b == nb - 1))
                    col += 1
        blockmeta.append(metas)
        gbase[g + 1] = col
    nblk = col
    nbgmax = int((gbase[1:] - gbase[:-1]).max())

    # per-core idx (pad n_nodes -> skipped) and trel (pad -1) tables
    idx_cores, trel_cores = [], []
    inv = np.empty_like(orders)
    for c in range(NCORE):
        inv[c][orders[c]] = np.arange(npos)
    for c in range(NCORE):
        idx = np.full(nblk * 128, n_nodes, dtype=np.int32)
        trel = np.full(nblk * 128, -1.0, dtype=np.float32)
        for d, pc in ((0, pc_p), (1, pc_n)):
            tl, sc, binid = pc[c]
            estart = np.zeros(npos + 1, dtype=np.int64)
            np.cumsum(np.bincount(binid, minlength=npos), out=estart[1:])
            pos = inv[c][binid]                   # processing position
            rank = np.arange(len(tl)) - estart[binid]
            flat = (first_col[d][pos] + rank // 128) * 128 + rank % 128
            idx[flat] = sc
            trel[flat] = (tl - binid * W).astype(np.float32)
        idx_cores.append(np.ascontiguousarray(idx.reshape(nblk, 128).T))
        trel_cores.append(np.ascontiguousarray(
            trel.reshape(nblk, 128).T.astype(BF)))

    x_bf = np.ascontiguousarray(x.astype(BF))
    wn = np.asarray(inputs["w_next"], dtype=np.float32).astype(BF)
    wpv = np.asarray(inputs["w_prev"], dtype=np.float32).astype(BF)
    gk = np.asarray(inputs["gru_kernel"], dtype=np.float32).astype(BF)
    gr = np.asarray(inputs["gru_rec_kernel"], dtype=np.float32).astype(BF)
    gb = np.asarray(inputs["gru_bias"], dtype=np.float32)
    bb = np.asarray(inputs["b"], dtype=np.float32).reshape(-1)
    gamma = np.asarray(inputs["bn_gamma"], dtype=np.float32).reshape(-1)
    beta = np.asarray(inputs["bn_beta"], dtype=np.float32).reshape(-1)

    misc = np.zeros((128, 8), dtype=np.float32)
    misc[:, 0] = bb
    misc[:, 1] = gb[0, 0:F] + gb[1, 0:F]          # z bias
    misc[:, 2] = gb[0, F:2 * F] + gb[1, F:2 * F]  # r bias
    misc[:, 3] = gb[0, 2 * F:3 * F]               # h kernel bias
    misc[:, 4] = gb[1, 2 * F:3 * F]               # h recurrent bias
    misc[:, 5] = gamma
    misc[:, 6] = beta
    misc[:, 7] = BN_EPS

    jt_all = np.ascontiguousarray(np.broadcast_to(
        np.arange(W, dtype=np.float32), (128, W)).astype(BF))
    ident = np.eye(F, dtype=np.float32).astype(BF)

    # per-core xT in processing order
    colmaps = []
    in_maps = []
    for c in range(NCORE):
        cm = np.concatenate([
            (orders[c, :npos - 1, None] * W
             + np.arange(W)[None, :]).ravel(),
            np.arange((npos - 1) * W, ncn),
        ])
        colmaps.append(cm)
        xs_t = x[c * ncn:(c + 1) * ncn].T        # [128, ncn] f32
        xt_proc = np.ascontiguousarray(xs_t[:, cm].astype(BF))
        in_maps.append({
            "x_bf": x_bf,
            "xT": xt_proc,
            "idx_all": idx_cores[c],
            "trel_all": trel_cores[c],
            "w_prev": wpv, "w_next": wn,
            "gru_kernel": gk, "gru_rec": gr,
            "jt_all": jt_all, "ident": ident, "misc": misc,
        })

    meta = dict(n_nodes=n_nodes, ncn=ncn, npos=npos, wlast=wlast,
                ngroups=ngroups, nblk=nblk, nbgmax=nbgmax,
                gbase=gbase, blockmeta=blockmeta)
    return meta, in_maps, colmaps


# ---------------------------------------------------------------- program

def _build_program(meta):
    n_nodes = meta["n_nodes"]
    ncn = meta["ncn"]
    npos = meta["npos"]
    wlast = meta["wlast"]
    ngroups = meta["ngroups"]
    nblk = meta["nblk"]
    nbgmax = meta["nbgmax"]
    gbase = meta["gbase"]
    blockmeta = meta["blockmeta"]
    GW = GP * W                                   # group width (512)

    nc = bass.Bass()
    x_d = nc.dram_tensor("x_bf", [n_nodes, F], f16, kind="ExternalInput")
    xt_d = nc.dram_tensor("xT", [F, ncn], f16, kind="ExternalInput")
    idx_d = nc.dram_tensor("idx_all", [128, nblk], i32, kind="ExternalInput")
    trl_d = nc.dram_tensor("trel_all", [128, nblk], f16, kind="ExternalInput")
    wp_d = nc.dram_tensor("w_prev", [F, F], f16, kind="ExternalInput")
    wn_d = nc.dram_tensor("w_next", [F, F], f16, kind="ExternalInput")
    gk_d = nc.dram_tensor("gru_kernel", [F, 3 * F], f16, kind="ExternalInput")
    gr_d = nc.dram_tensor("gru_rec", [F, 3 * F], f16, kind="ExternalInput")
    jt_d = nc.dram_tensor("jt_all", [128, W], f16, kind="ExternalInput")
    id_d = nc.dram_tensor("ident", [F, F], f16, kind="ExternalInput")
    ms_d = nc.dram_tensor("misc", [128, 8], f32, kind="ExternalInput")
    out_d = nc.dram_tensor("outT", [F, ncn], f16, kind="ExternalOutput")
    sin_d = nc.dram_tensor("stats_in", [128, 2], f32)
    sout_d = nc.dram_tensor("stats_out", [128, 2], f32, addr_space="Shared")

    with tile.TileContext(nc) as tc:
        with (
            tc.tile_pool(name="const", bufs=1) as cp,
            tc.tile_pool(name="stats", bufs=1) as stp,
            tc.tile_pool(name="arest", bufs=1) as ap_,
        ):
            idxt = cp.tile([128, nblk], i32)
            trlt = cp.tile([128, nblk], f16)
            wp = cp.tile([F, F], f16)
            wn = cp.tile([F, F], f16)
            gkw = cp.tile([F, 3 * F], f16)
            grw = cp.tile([F, 3 * F], f16)
            ident = cp.tile([F, F], f16)
            jt = cp.tile([128, W], f16)
            ms = cp.tile([128, 8], f32)
            for sb, d in [(idxt, idx_d), (trlt, trl_d),
                          (wp, wp_d), (wn, wn_d),
                          (gkw, gk_d), (grw, gr_d),
                          (ident, id_d), (jt, jt_d), (ms, ms_d)]:
                nc.sync.dma_start(sb[:], d[:])

            A = ap_.tile([128, ncn], f16)           # resident relu(aggre+b)
            st_sum = stp.tile([128, ngroups], f32)
            st_sq = stp.tile([128, ngroups], f32)

            # ---------------- pass 1
            GB = 2                                   # gather bufs
            breg = nc.gpsimd.to_reg(n_nodes - 1)     # shared bounds register
            with (
                tc.tile_pool(name="gat", bufs=GB) as gp_,
                tc.tile_pool(name="pone", bufs=2) as pp,
                tc.tile_pool(name="ssb", bufs=2) as sp,
                tc.tile_pool(name="io1", bufs=3) as iop,
                tc.tile_pool(name="sqp", bufs=2) as qp,
                tc.tile_pool(name="psum1", bufs=2, space="PSUM") as ps1,
            ):
                for g in range(ngroups):
                    nb = int(gbase[g + 1] - gbase[g])
                    b0 = int(gbase[g])
                    metas = blockmeta[g]
                    off = g * GW
                    wg = min(GW, ncn - off)
                    npg = (wg + W - 1) // W          # positions in group
                    xt = iop.tile([128, GW], f16, tag="xt")
                    nc.sync.dma_start(xt[:, 0:wg], xt_d[:, off:off + wg])

                    G = gp_.tile([128, nbgmax * 128], f16, tag="G")
                    if g < GB:
                        nc.gpsimd.memset(G[:], 0.0)
                    for lb in range(nb):
                        nc.gpsimd.indirect_dma_start(
                            out=G[:, lb * 128:(lb + 1) * 128],
                            out_offset=None,
                            in_=x_d[:],
                            in_offset=IndirectOffsetOnAxis(
                                ap=idxt[:, b0 + lb:b0 + lb + 1], axis=0),
                            bounds_check=breg,
                            oob_is_err=False,
                        )
                    P = pp.tile([128, nbgmax * W], f16, tag="P")
                    nc.vector.tensor_tensor(
                        out=P[:, 0:nb * W],
                        in0=jt[:].unsqueeze(1).broadcast_to([128, nb, W]),
                        in1=trlt[:, b0:b0 + nb].unsqueeze(2)
                            .broadcast_to([128, nb, W]),
                        op=ALU.is_equal,
                    )
                    stq = ps1.tile([128, 2 * GW], f32, tag="stq")
                    for k, (colk, soff, st, sp_) in enumerate(metas):
                        lb = colk - b0
                        nc.tensor.matmul(
                            stq[:, soff:soff + W],
                            G[:, lb * 128:(lb + 1) * 128],
                            P[:, lb * W:(lb + 1) * W],
                            start=st, stop=sp_)
                    S = sp.tile([128, 2 * GW], f16, tag="S")
                    if wg == GW:
                        nc.vector.tensor_copy(S[:, 0:GW], stq[:, 0:GW])
                        nc.scalar.copy(S[:, GW:2 * GW], stq[:, GW:2 * GW])
                    else:
                        nc.vector.tensor_copy(S[:, 0:wg], stq[:, 0:wg])
                        nc.scalar.copy(S[:, GW:GW + wg], stq[:, GW:GW + wg])
                    agg = ps1.tile([128, GW], f32, tag="agg")
                    nc.tensor.matmul(agg[:, 0:wg], wp[:], S[:, 0:wg],
                                     start=True, stop=False)
                    nc.tensor.matmul(agg[:, 0:wg], wn[:], S[:, GW:GW + wg],
                                     start=False, stop=False)
                    nc.tensor.matmul(agg[:, 0:wg], ident[:], xt[:, 0:wg],
                                     start=False, stop=True)
                    nc.scalar.activation(A[:, off:off + wg], agg[:, 0:wg],
                                         AF.Relu, bias=ms[:, 0:1],
                                         accum_out=st_sum[:, g:g + 1])
                    scr = qp.tile([128, GW], f16, tag="scr")
                    nc.scalar.activation(
                        scr[:, 0:wg], A[:, off:off + wg], AF.Square,
                        accum_out=st_sq[:, g:g + 1])

            # ---------------- global BN stats + folded weights/biases
            red = stp.tile([128, 2], f32)
            nc.vector.reduce_sum(red[:, 0:1], st_sum[:],
                                 axis=mybir.AxisListType.X)
            nc.vector.reduce_sum(red[:, 1:2], st_sq[:],
                                 axis=mybir.AxisListType.X)
            tot = stp.tile([128, 2], f32)
            if NCORE > 1:
                nc.sync.dma_start(sin_d[:], red[:])
                nc.gpsimd.collective_compute(
                    "AllReduce", ALU.add,
                    replica_groups=[list(range(NCORE))],
                    ins=[sin_d[:]], outs=[sout_d[:]])
                nc.sync.dma_start(tot[:], sout_d[:])
            else:
                nc.vector.tensor_copy(tot[:], red[:])
            inv_n = 1.0 / float(n_nodes)
            mcol = stp.tile([128, 1], f32)
            nc.vector.tensor_scalar_mul(mcol[:], tot[:, 0:1], inv_n)
            ecol = stp.tile([128, 1], f32)
            nc.vector.tensor_scalar_mul(ecol[:], tot[:, 1:2], inv_n)
            msq = stp.tile([128, 1], f32)
            nc.vector.tensor_mul(msq[:], mcol[:], mcol[:])
            var = stp.tile([128, 1], f32)
            nc.vector.tensor_sub(var[:], ecol[:], msq[:])
            sd = stp.tile([128, 1], f32)
            nc.scalar.activation(sd[:], var[:], AF.Sqrt, bias=ms[:, 7:8])
            rstd = stp.tile([128, 1], f32)
            nc.vector.reciprocal(rstd[:], sd[:])
            scol = stp.tile([128, 1], f32)
            nc.vector.tensor_mul(scol[:], rstd[:], ms[:, 5:6])
            tmp = stp.tile([128, 1], f32)
            nc.vector.tensor_mul(tmp[:], mcol[:], scol[:])
            shcol = stp.tile([128, 1], f32)
            nc.vector.tensor_sub(shcol[:], ms[:, 6:7], tmp[:])
            shb = stp.tile([128, 1], f16)
            nc.vector.tensor_copy(shb[:], shcol[:])
            # fold scol into x-side GRU kernel (per in-feature row scale)
            gks = stp.tile([F, 3 * F], f16)
            nc.vector.tensor_scalar_mul(gks[:], gkw[:], scol[:])
            # fold shcol into gate biases: bias_g += gk_g^T @ shcol
            with tc.tile_pool(name="psb", bufs=1, space="PSUM") as psb:
                zc = psb.tile([128, 1], f32)
                rc = psb.tile([128, 1], f32)
                hcc = psb.tile([128, 1], f32)
                nc.tensor.matmul(zc[:], gkw[:, 0:F], shb[:],
                                 start=True, stop=True)
                nc.tensor.matmul(rc[:], gkw[:, F:2 * F], shb[:],
                                 start=True, stop=True)
                nc.tensor.matmul(hcc[:], gkw[:, 2 * F:3 * F], shb[:],
                                 start=True, stop=True)
                bz = stp.tile([128, 1], f32)
                br = stp.tile([128, 1], f32)
                bh = stp.tile([128, 1], f32)
                nc.vector.tensor_add(bz[:], ms[:, 1:2], zc[:])
                nc.vector.tensor_add(br[:], ms[:, 2:3], rc[:])
                nc.vector.tensor_add(bh[:], ms[:, 3:4], hcc[:])

            # ---------------- pass 2: GRU
            CH = 1536
            nch = (ncn + CH - 1) // CH
            with (
                tc.tile_pool(name="p2", bufs=2) as p2,
                tc.tile_pool(name="p2io", bufs=2) as p2io,
                tc.tile_pool(name="psum2", bufs=2, space="PSUM") as ps2,
            ):
                for ci in range(nch):
                    o = ci * CH
                    wc = min(CH, ncn - o)
                    nj = (wc + 511) // 512
                    xt2 = p2io.tile([128, wc], f16, tag="xt2")
                    nc.sync.dma_start(xt2[:], xt_d[:, o:o + wc])

                    def mmq(qt, k1, rhs1, k2=None, rhs2=None, k3=None,
                            rhs3=None, stop=True):
                        for j in range(nj):
                            js = slice(j * 512, min((j + 1) * 512, wc))
                            nc.tensor.matmul(qt[:, js], k1, rhs1[:, js],
                                             start=True,
                                             stop=(k2 is None and stop))
                        if k2 is not None:
                            for j in range(nj):
                                js = slice(j * 512, min((j + 1) * 512, wc))
                                nc.tensor.matmul(
                                    qt[:, js], k2, rhs2[:, js],
                                    start=False,
                                    stop=(k3 is None and stop))
                        if k3 is not None:
                            for j in range(nj):
                                js = slice(j * 512, min((j + 1) * 512, wc))
                                nc.tensor.matmul(qt[:, js], k3, rhs3[:, js],
                                                 start=False, stop=stop)

                    # r gate
                    rp = ps2.tile([128, CH], f32, tag="q")
                    mmq(rp, gks[:, F:2 * F], A[:, o:o + wc],
                        grw[:, F:2 * F], xt2)
                    r = p2.tile([128, wc], f16, tag="r")
                    nc.scalar.activation(r[:], rp[:, 0:wc], AF.Sigmoid, bias=br[:])
                    # gq = gr_h^T xt
                    gq = ps2.tile([128, CH], f32, tag="q")
                    mmq(gq, grw[:, 2 * F:3 * F], xt2)
                    u = p2.tile([128, wc], f16, tag="u")
                    nc.vector.scalar_tensor_tensor(
                        out=u[:], in0=gq[:, 0:wc], scalar=ms[:, 4:5], in1=r[:],
                        op0=ALU.add, op1=ALU.mult)
                    # hp = gk_h^T A + I u  (t2 folded into PSUM)
                    hp = ps2.tile([128, CH], f32, tag="q")
                    mmq(hp, gks[:, 2 * F:3 * F], A[:, o:o + wc],
                        ident[:], u)
                    hc = p2.tile([128, wc], f16, tag="hc")
                    nc.scalar.activation(hc[:], hp[:, 0:wc], AF.Tanh, bias=bh[:])
                    # z gate
                    zp = ps2.tile([128, CH], f32, tag="q")
                    mmq(zp, gks[:, 0:F], A[:, o:o + wc], grw[:, 0:F], xt2)
                    z = p2.tile([128, wc], f16, tag="z")
                    nc.scalar.activation(z[:], zp[:, 0:wc], AF.Sigmoid, bias=bz[:])
                    dd = p2.tile([128, wc], f16, tag="dd")
                    nc.vector.tensor_sub(dd[:], xt2[:], hc[:])
                    oo2 = p2.tile([128, wc], f16, tag="oo2")
                    nc.vector.tensor_mul(oo2[:], dd[:], z[:])
                    oo = p2.tile([128, wc], f16, tag="oo")
                    nc.vector.tensor_add(oo[:], oo2[:], hc[:])
                    nc.sync.dma_start(out_d[:, o:o + wc], oo[:])

    return nc


# ---------------------------------------------------------------- kernel

def kernel(**inputs):
    _install_ntff_hook()
    from concourse.bass_utils import run_bass_kernel_spmd
    meta, in_maps, colmaps = _prepare(inputs)
    nc = _build_program(meta)
    _split_excess_waits(nc, cap=1)
    trace = bool(int(os.environ.get("KERNEL_TRACE", "0")))
    kw = {}
    if trace:
        kw = dict(trace=True,
                  tmpdir=os.environ.get("KERNEL_TRACE_DIR",
                                        "/tmp/kernel_trace"))
    res = run_bass_kernel_spmd(nc, in_maps, list(range(NCORE)), **kw)
    if trace:
        kernel.last_exec_time_ns = res.exec_time_ns
    ncn = meta["ncn"]
    outs = []
    for c in range(NCORE):
        op = np.asarray(res.results[c]["outT"], dtype=np.float32)
        on = np.empty_like(op)
        on[:, colmaps[c]] = op
        outs.append(on.T)
    return np.concatenate(outs, axis=0)


kernel.last_exec_time_ns = None


# revision 13
# speedup vs baseline: 1.2212x; 1.2212x over previous
"""Trainium2 Bass kernel for nn_MessagePassingConv (GNN message passing) — v3.

Strategy (8 NeuronCores, SPMD):
- Nodes sharded by row range: core c owns target nodes [c*62500, (c+1)*62500).
- Edges sharded by TARGET core; x replicated (f16 copy) so source gathers are
  local. Only a [128,2] BN-stat AllReduce crosses cores.
- W=128 target windows; groups of GP=4 positions processed together.
  Per-core processing order sorts positions by edge count so the shared SPMD
  block counts B[i] track quantiles, not maxima.
- Per group: ONE batched indirect-DMA gather (all blocks, pads skipped via
  bounds_check), ONE broadcast is_equal builds all one-hot blocks, scatter
  matmuls (f16) accumulate S^T slices in a [128,1024] PSUM tile;
  aggre^T = wp^T Sp^T + wn^T Sn^T + I xt in PSUM; relu(aggre + b) lands in an
  SBUF-resident A [128, 62500] f16 with BN-sum accum; sq-sum via fused
  tensor_tensor_reduce on DVE.
- BN shift folded into GRU gate biases (3 tiny matmuls); BN scale folded into
  the GRU x-side kernel weights (one tensor_scalar) so pass 2 reads A raw.
- Pass 2 (GRU) in chunks of 1536 nodes: f16 matmuls; t2=hp+u folded into PSUM
  via identity matmul; sigmoid/tanh on ACT; dd/oo2/oo on DVE; output f16.
"""
import os
import sys
import types
import contextlib

import numpy as np

import concourse.bass as bass
import concourse.mybir as mybir
import concourse.tile as tile
from concourse.bass import IndirectOffsetOnAxis
from concourse.vector_clock import ScopedClock

F = 128
NCORE = 8
W = 512          # scatter window (targets per position)
GP = 1           # positions per group
BN_EPS = 1e-3
AF = mybir.ActivationFunctionType
ALU = mybir.AluOpType
f32 = mybir.dt.float32
f16 = mybir.dt.float16
i32 = mybir.dt.int32
BF = np.float16

# ---------------------------------------------------------------- patches

_MAXW = 1


def _patched_drain_and_barrier(self, tick_clock, wait_clock):
    nc = self.nc
    drain_inst = nc.sync.drain()
    wait_clock.add_sem_waits(
        drain_inst.ins, ScopedClock({None: tick_clock.global_clock})
    )
    si = drain_inst.ins.sync_info
    waits = list(si.on_wait) if si and si.on_wait else []
    if len(waits) > _MAXW:
        drain_inst.ins.sync_info = mybir.SyncInfo(
            on_wait=waits[:_MAXW], on_update=list(si.on_update or []))
        for i in range(_MAXW, len(waits), _MAXW):
            extra = nc.sync.drain()
            extra.ins.sync_info = mybir.SyncInfo(
                on_wait=waits[i:i + _MAXW], on_update=[])
    nc.all_engine_barrier()
    assert self.sems is not None
    popped = nc._tile_sem_poison_stack.pop()
    assert popped is self._sem_poison
    nc.clear_and_free_semaphores(list(self.sems.allocated().values()))
    nc.all_engine_barrier()


tile.TileContext._drain_and_barrier = _patched_drain_and_barrier

_split_n = [0]


def _split_excess_waits(nc, cap=1):
    """Walrus codegen caps sync-waits per instruction; move excess onto
    preceding same-engine InstNoOps."""
    for f in nc.m.functions:
        for blk in f.blocks:
            insts = blk.instructions
            if not any(i.sync_info and i.sync_info.on_wait
                       and len(i.sync_info.on_wait) > cap for i in insts):
                continue
            new = []
            for inst in insts:
                si = inst.sync_info
                waits = list(si.on_wait) if si and si.on_wait else []
                if len(waits) > cap:
                    keep, excess = waits[:cap], waits[cap:]
                    for j in range(0, len(excess), cap):
                        _split_n[0] += 1
                        nop = mybir.InstNoOp(
                            name=f"waitsplit-{_split_n[0]}",
                            ins=[], outs=[], engine=inst.engine)
                        nop.sync_info = mybir.SyncInfo(
                            on_wait=excess[j:j + cap], on_update=[])
                        new.append(nop)
                    inst.sync_info = mybir.SyncInfo(
                        on_wait=keep, on_update=list(si.on_update or []))
                new.append(inst)
            blk.instructions = new


def _install_ntff_hook():
    """Provide antenv.axon_hooks (missing in image) so trace=True works."""
    import ctypes
    if "antenv.axon_hooks" in sys.modules:
        return
    try:
        lib = ctypes.CDLL("/opt/axon/libaxon_pjrt.so")
    except OSError:
        return
    if not hasattr(lib, "axon_start_nrt_profile"):
        return
    lib.axon_start_nrt_profile.argtypes = [
        ctypes.POINTER(ctypes.c_int64), ctypes.c_size_t]
    lib.axon_start_nrt_profile.restype = ctypes.c_int64
    lib.axon_stop_nrt_profile.argtypes = [ctypes.c_char_p]
    lib.axon_stop_nrt_profile.restype = ctypes.c_int64

    @contextlib.contextmanager
    def _hook(output_dir, device_ids):
        import jax
        jax.devices()
        if device_ids:
            ids = (ctypes.c_int64 * len(device_ids))(*device_ids)
            rc = lib.axon_start_nrt_profile(ids, len(device_ids))
        else:
            rc = lib.axon_start_nrt_profile(None, 0)
        if rc != 0:
            raise RuntimeError(f"axon_start_nrt_profile rc={rc}")
        try:
            yield
        finally:
            n = lib.axon_stop_nrt_profile(str(output_dir).encode())
            print(f"profile: {n} file(s) written to {output_dir}")

    mod = types.ModuleType("antenv.axon_hooks")
    mod.get_axon_ntff_profile_hook = lambda: _hook
    mod.set_axon_ntff_profile_hook = lambda h: None
    import antenv
    sys.modules["antenv.axon_hooks"] = mod
    antenv.axon_hooks = mod
    from concourse import bass_utils
    bass_utils.upload_artifacts = lambda tmpdir: tmpdir


# ---------------------------------------------------------------- host prep

def _edge_layout(pairs, ncn, npos):
    """Per-core (src sorted by (bin, src), estart) and cnt [NCORE, npos]."""
    t = np.asarray(pairs[:, 0], dtype=np.int64)
    s = np.asarray(pairs[:, 1], dtype=np.int64)
    per_core = []
    cnt = np.zeros((NCORE, npos), dtype=np.int64)
    for c in range(NCORE):
        m = (t >= c * ncn) & (t < (c + 1) * ncn)
        tl = t[m] - c * ncn
        sc = s[m]
        binid = tl // W
        order = np.lexsort((sc, binid))      # bin-major, src-minor
        tl = tl[order]
        sc = sc[order]
        binid = binid[order]
        cnt[c] = np.bincount(binid, minlength=npos)
        estart = np.zeros(npos + 1, dtype=np.int64)
        np.cumsum(cnt[c], out=estart[1:])
        per_core.append((tl, sc, binid))
    return per_core, cnt


def _prepare(inputs):
    x = np.asarray(inputs["x"], dtype=np.float32)
    n_nodes = x.shape[0]
    ncn = n_nodes // NCORE
    npos = (ncn + W - 1) // W                     # 489
    wlast = ncn - (npos - 1) * W                  # 36

    pc_p, cnt_p = _edge_layout(inputs["pairs_prev"], ncn, npos)
    pc_n, cnt_n = _edge_layout(inputs["pairs_next"], ncn, npos)

    # per-core processing order: full bins sorted lexicographically by
    # (ceil_p/128, ceil_n/128, total) so per-direction block counts align
    # across cores at each rank; partial bin last
    orders = np.zeros((NCORE, npos), dtype=np.int64)
    for c in range(NCORE):
        bp = -(-cnt_p[c, :npos - 1] // 128)
        bn = -(-cnt_n[c, :npos - 1] // 128)
        key = ((bp * 64 + bn) * 100000
               + cnt_p[c, :npos - 1] + cnt_n[c, :npos - 1])
        orders[c, :npos - 1] = np.argsort(key, kind="stable")
        orders[c, npos - 1] = npos - 1

    # shared (SPMD) block counts per processing position, per direction
    def blocks(cnt):
        per_pos = np.take_along_axis(cnt, orders, axis=1)  # [NCORE, npos]
        return np.maximum(1, -(-per_pos // 128)).max(axis=0)  # [npos]

    Bd = [blocks(cnt_p), blocks(cnt_n)]           # [2][npos]

    # group structure: GP positions per group, block table layout
    # order: group -> position-in-group -> dir -> block
    ngroups = (npos + GP - 1) // GP
    first_col = np.zeros((2, npos), dtype=np.int64)
    blockmeta = []       # per group: list of (col, slice_off, start, stop)
    gbase = np.zeros(ngroups + 1, dtype=np.int64)
    col = 0
    for g in range(ngroups):
        metas = []
        for il, i in enumerate(range(g * GP, min((g + 1) * GP, npos))):
            for d in range(2):
                nb = int(Bd[d][i])
                first_col[d, i] = col
                for b in range(nb):
                    metas.append((col, d * (GP * W) + il * W,
                                  b == 0, b == nb - 1))
                    col += 1
        blockmeta.append(metas)
        gbase[g + 1] = col
    nblk = col
    nbgmax = int((gbase[1:] - gbase[:-1]).max())

    # per-core idx (pad n_nodes -> skipped) and trel (pad -1) tables
    idx_cores, trel_cores = [], []
    inv = np.empty_like(orders)
    for c in range(NCORE):
        inv[c][orders[c]] = np.arange(npos)
    for c in range(NCORE):
        idx = np.zeros(nblk * 128, dtype=np.int32)
        trel = np.full(nblk * 128, -1.0, dtype=np.float32)
        for d, pc in ((0, pc_p), (1, pc_n)):
            tl, sc, binid = pc[c]
            estart = np.zeros(npos + 1, dtype=np.int64)
            np.cumsum(np.bincount(binid, minlength=npos), out=estart[1:])
            pos = inv[c][binid]                   # processing position
            rank = np.arange(len(tl)) - estart[binid]
            flat = (first_col[d][pos] + rank // 128) * 128 + rank % 128
            idx[flat] = sc
            trel[flat] = (tl - binid * W).astype(np.float32)
        idx_cores.append(np.ascontiguousarray(idx.reshape(nblk, 128).T))
        trel_cores.append(np.ascontiguousarray(
            trel.reshape(nblk, 128).T.astype(BF)))

    x_bf = np.ascontiguousarray(x.astype(BF))
    wn = np.asarray(inputs["w_next"], dtype=np.float32).astype(BF)
    wpv = np.asarray(inputs["w_prev"], dtype=np.float32).astype(BF)
    gk = np.asarray(inputs["gru_kernel"], dtype=np.float32).astype(BF)
    gr = np.asarray(inputs["gru_rec_kernel"], dtype=np.float32).astype(BF)
    gb = np.asarray(inputs["gru_bias"], dtype=np.float32)
    bb = np.asarray(inputs["b"], dtype=np.float32).reshape(-1)
    gamma = np.asarray(inputs["bn_gamma"], dtype=np.float32).reshape(-1)
    beta = np.asarray(inputs["bn_beta"], dtype=np.float32).reshape(-1)

    misc = np.zeros((128, 8), dtype=np.float32)
    misc[:, 0] = bb
    misc[:, 1] = gb[0, 0:F] + gb[1, 0:F]          # z bias
    misc[:, 2] = gb[0, F:2 * F] + gb[1, F:2 * F]  # r bias
    misc[:, 3] = gb[0, 2 * F:3 * F]               # h kernel bias
    misc[:, 4] = gb[1, 2 * F:3 * F]               # h recurrent bias
    misc[:, 5] = gamma
    misc[:, 6] = beta
    misc[:, 7] = BN_EPS

    jt_all = np.ascontiguousarray(np.broadcast_to(
        np.arange(W, dtype=np.float32), (128, W)).astype(BF))
    ident = np.eye(F, dtype=np.float32).astype(BF)

    # per-core xT in processing order
    colmaps = []
    in_maps = []
    for c in range(NCORE):
        cm = np.concatenate([
            (orders[c, :npos - 1, None] * W
             + np.arange(W)[None, :]).ravel(),
            np.arange((npos - 1) * W, ncn),
        ])
        colmaps.append(cm)
        xs_t = x[c * ncn:(c + 1) * ncn].T        # [128, ncn] f32
        xt_proc = np.ascontiguousarray(xs_t[:, cm].astype(BF))
        in_maps.append({
            "x_bf": x_bf,
            "xT": xt_proc,
            "idx_all": idx_cores[c],
            "trel_all": trel_cores[c],
            "w_prev": wpv, "w_next": wn,
            "gru_kernel": gk, "gru_rec": gr,
            "jt_all": jt_all, "ident": ident, "misc": misc,
        })

    meta = dict(n_nodes=n_nodes, ncn=ncn, npos=npos, wlast=wlast,
                ngroups=ngroups, nblk=nblk, nbgmax=nbgmax,
                gbase=gbase, blockmeta=blockmeta)
    return meta, in_maps, colmaps


# ---------------------------------------------------------------- program

def _build_program(meta):
    n_nodes = meta["n_nodes"]
    ncn = meta["ncn"]
    npos = meta["npos"]
    wlast = meta["wlast"]
    ngroups = meta["ngroups"]
    nblk = meta["nblk"]
    nbgmax = meta["nbgmax"]
    gbase = meta["gbase"]
    blockmeta = meta["blockmeta"]
    GW = GP * W                                   # group width (512)

    nc = bass.Bass()
    x_d = nc.dram_tensor("x_bf", [n_nodes, F], f16, kind="ExternalInput")
    xt_d = nc.dram_tensor("xT", [F, ncn], f16, kind="ExternalInput")
    idx_d = nc.dram_tensor("idx_all", [128, nblk], i32, kind="ExternalInput")
    trl_d = nc.dram_tensor("trel_all", [128, nblk], f16, kind="ExternalInput")
    wp_d = nc.dram_tensor("w_prev", [F, F], f16, kind="ExternalInput")
    wn_d = nc.dram_tensor("w_next", [F, F], f16, kind="ExternalInput")
    gk_d = nc.dram_tensor("gru_kernel", [F, 3 * F], f16, kind="ExternalInput")
    gr_d = nc.dram_tensor("gru_rec", [F, 3 * F], f16, kind="ExternalInput")
    jt_d = nc.dram_tensor("jt_all", [128, W], f16, kind="ExternalInput")
    id_d = nc.dram_tensor("ident", [F, F], f16, kind="ExternalInput")
    ms_d = nc.dram_tensor("misc", [128, 8], f32, kind="ExternalInput")
    out_d = nc.dram_tensor("outT", [F, ncn], f16, kind="ExternalOutput")
    sin_d = nc.dram_tensor("stats_in", [128, 2], f32)
    sout_d = nc.dram_tensor("stats_out", [128, 2], f32, addr_space="Shared")

    with tile.TileContext(nc) as tc:
        with (
            tc.tile_pool(name="const", bufs=1) as cp,
            tc.tile_pool(name="stats", bufs=1) as stp,
            tc.tile_pool(name="arest", bufs=1) as ap_,
        ):
            idxt = cp.tile([128, nblk], i32)
            trlt = cp.tile([128, nblk], f16)
            wp = cp.tile([F, F], f16)
            wn = cp.tile([F, F], f16)
            gkw = cp.tile([F, 3 * F], f16)
            grw = cp.tile([F, 3 * F], f16)
            ident = cp.tile([F, F], f16)
            jt = cp.tile([128, W], f16)
            ms = cp.tile([128, 8], f32)
            for sb, d in [(idxt, idx_d), (trlt, trl_d),
                          (wp, wp_d), (wn, wn_d),
                          (gkw, gk_d), (grw, gr_d),
                          (ident, id_d), (jt, jt_d), (ms, ms_d)]:
                nc.sync.dma_start(sb[:], d[:])

            A = ap_.tile([128, ncn], f16)           # resident relu(aggre+b)
            st_sum = stp.tile([128, ngroups], f32)
            st_sq = stp.tile([128, ngroups], f32)

            # ---------------- pass 1
            GB = 3                                   # gather bufs
            with (
                tc.tile_pool(name="gat", bufs=GB) as gp_,
                tc.tile_pool(name="pone", bufs=3) as pp,
                tc.tile_pool(name="ssb", bufs=2) as sp,
                tc.tile_pool(name="io1", bufs=3) as iop,
                tc.tile_pool(name="sqp", bufs=2) as qp,
                tc.tile_pool(name="psum1", bufs=2, space="PSUM") as ps1,
            ):
                for g in range(ngroups):
                    nb = int(gbase[g + 1] - gbase[g])
                    b0 = int(gbase[g])
                    metas = blockmeta[g]
                    off = g * GW
                    wg = min(GW, ncn - off)
                    npg = (wg + W - 1) // W          # positions in group
                    xt = iop.tile([128, GW], f16, tag="xt")
                    nc.sync.dma_start(xt[:, 0:wg], xt_d[:, off:off + wg])

                    G = gp_.tile([128, nbgmax * 128], f16, tag="G")
                    for lb in range(nb):
                        nc.gpsimd.indirect_dma_start(
                            out=G[:, lb * 128:(lb + 1) * 128],
                            out_offset=None,
                            in_=x_d[:],
                            in_offset=IndirectOffsetOnAxis(
                                ap=idxt[:, b0 + lb:b0 + lb + 1], axis=0),
                        )
                    P = pp.tile([128, nbgmax * W], f16, tag="P")
                    nc.vector.tensor_tensor(
                        out=P[:, 0:nb * W],
                        in0=jt[:].unsqueeze(1).broadcast_to([128, nb, W]),
                        in1=trlt[:, b0:b0 + nb].unsqueeze(2)
                            .broadcast_to([128, nb, W]),
                        op=ALU.is_equal,
                    )
                    stq = ps1.tile([128, 2 * GW], f32, tag="stq")
                    for k, (colk, soff, st, sp_) in enumerate(metas):
                        lb = colk - b0
                        nc.tensor.matmul(
                            stq[:, soff:soff + W],
                            G[:, lb * 128:(lb + 1) * 128],
                            P[:, lb * W:(lb + 1) * W],
                            start=st, stop=sp_)
                    S = sp.tile([128, 2 * GW], f16, tag="S")
                    if wg == GW:
                        nc.vector.tensor_copy(S[:, 0:GW], stq[:, 0:GW])
                        nc.scalar.copy(S[:, GW:2 * GW], stq[:, GW:2 * GW])
                    else:
                        nc.vector.tensor_copy(S[:, 0:wg], stq[:, 0:wg])
                        nc.scalar.copy(S[:, GW:GW + wg], stq[:, GW:GW + wg])
                    agg = ps1.tile([128, GW], f32, tag="agg")
                    nc.tensor.matmul(agg[:, 0:wg], wp[:], S[:, 0:wg],
                                     start=True, stop=False)
                    nc.tensor.matmul(agg[:, 0:wg], wn[:], S[:, GW:GW + wg],
                                     start=False, stop=False)
                    nc.tensor.matmul(agg[:, 0:wg], ident[:], xt[:, 0:wg],
                                     start=False, stop=True)
                    nc.scalar.activation(A[:, off:off + wg], agg[:, 0:wg],
                                         AF.Relu, bias=ms[:, 0:1],
                                         accum_out=st_sum[:, g:g + 1])
                    scr = qp.tile([128, GW], f16, tag="scr")
                    nc.scalar.activation(
                        scr[:, 0:wg], A[:, off:off + wg], AF.Square,
                        accum_out=st_sq[:, g:g + 1])

            # ---------------- global BN stats + folded weights/biases
            red = stp.tile([128, 2], f32)
            nc.vector.reduce_sum(red[:, 0:1], st_sum[:],
                                 axis=mybir.AxisListType.X)
            nc.vector.reduce_sum(red[:, 1:2], st_sq[:],
                                 axis=mybir.AxisListType.X)
            tot = stp.tile([128, 2], f32)
            if NCORE > 1:
                nc.sync.dma_start(sin_d[:], red[:])
                nc.gpsimd.collective_compute(
                    "AllReduce", ALU.add,
                    replica_groups=[list(range(NCORE))],
                    ins=[sin_d[:]], outs=[sout_d[:]])
                nc.sync.dma_start(tot[:], sout_d[:])
            else:
                nc.vector.tensor_copy(tot[:], red[:])
            inv_n = 1.0 / float(n_nodes)
            mcol = stp.tile([128, 1], f32)
            nc.vector.tensor_scalar_mul(mcol[:], tot[:, 0:1], inv_n)
            ecol = stp.tile([128, 1], f32)
            nc.vector.tensor_scalar_mul(ecol[:], tot[:, 1:2], inv_n)
            msq = stp.tile([128, 1], f32)
            nc.vector.tensor_mul(msq[:], mcol[:], mcol[:])
            var = stp.tile([128, 1], f32)
            nc.vector.tensor_sub(var[:], ecol[:], msq[:])
            sd = stp.tile([128, 1], f32)
            nc.scalar.activation(sd[:], var[:], AF.Sqrt, bias=ms[:, 7:8])
            rstd = stp.tile([128, 1], f32)
            nc.vector.reciprocal(rstd[:], sd[:])
            scol = stp.tile([128, 1], f32)
            nc.vector.tensor_mul(scol[:], rstd[:], ms[:, 5:6])
            tmp = stp.tile([128, 1], f32)
            nc.vector.tensor_mul(tmp[:], mcol[:], scol[:])
            shcol = stp.tile([128, 1], f32)
            nc.vector.tensor_sub(shcol[:], ms[:, 6:7], tmp[:])
            shb = stp.tile([128, 1], f16)
            nc.vector.tensor_copy(shb[:], shcol[:])
            # fold scol into x-side GRU kernel (per in-feature row scale)
            gks = stp.tile([F, 3 * F], f16)
            nc.vector.tensor_scalar_mul(gks[:], gkw[:], scol[:])
            # fold shcol into gate biases: bias_g += gk_g^T @ shcol
            with tc.tile_pool(name="psb", bufs=1, space="PSUM") as psb:
                zc = psb.tile([128, 1], f32)
                rc = psb.tile([128, 1], f32)
                hcc = psb.tile([128, 1], f32)
                nc.tensor.matmul(zc[:], gkw[:, 0:F], shb[:],
                                 start=True, stop=True)
                nc.tensor.matmul(rc[:], gkw[:, F:2 * F], shb[:],
                                 start=True, stop=True)
                nc.tensor.matmul(hcc[:], gkw[:, 2 * F:3 * F], shb[:],
                                 start=True, stop=True)
                bz = stp.tile([128, 1], f32)
                br = stp.tile([128, 1], f32)
                bh = stp.tile([128, 1], f32)
                nc.vector.tensor_add(bz[:], ms[:, 1:2], zc[:])
                nc.vector.tensor_add(br[:], ms[:, 2:3], rc[:])
                nc.vector.tensor_add(bh[:], ms[:, 3:4], hcc[:])

            # ---------------- pass 2: GRU
            CH = 1536
            nch = (ncn + CH - 1) // CH
            with (
                tc.tile_pool(name="p2", bufs=2) as p2,
                tc.tile_pool(name="p2io", bufs=2) as p2io,
                tc.tile_pool(name="psum2", bufs=2, space="PSUM") as ps2,
            ):
                for ci in range(nch):
                    o = ci * CH
                    wc = min(CH, ncn - o)
                    nj = (wc + 511) // 512
                    xt2 = p2io.tile([128, wc], f16, tag="xt2")
                    nc.sync.dma_start(xt2[:], xt_d[:, o:o + wc])

                    def mmq(qt, k1, rhs1, k2=None, rhs2=None, k3=None,
                            rhs3=None, stop=True):
                        for j in range(nj):
                            js = slice(j * 512, min((j + 1) * 512, wc))
                            nc.tensor.matmul(qt[:, js], k1, rhs1[:, js],
                                             start=True,
                                             stop=(k2 is None and stop))
                        if k2 is not None:
                            for j in range(nj):
                                js = slice(j * 512, min((j + 1) * 512, wc))
                                nc.tensor.matmul(
                                    qt[:, js], k2, rhs2[:, js],
                                    start=False,
                                    stop=(k3 is None and stop))
                        if k3 is not None:
                            for j in range(nj):
                                js = slice(j * 512, min((j + 1) * 512, wc))
                                nc.tensor.matmul(qt[:, js], k3, rhs3[:, js],
                                                 start=False, stop=stop)

                    # r and z gate pre-activations first (independent)
                    rp = ps2.tile([128, CH], f32, tag="q")
                    mmq(rp, gks[:, F:2 * F], A[:, o:o + wc],
                        grw[:, F:2 * F], xt2)
                    r = p2.tile([128, wc], f16, tag="r")
                    nc.scalar.activation(r[:], rp[:, 0:wc], AF.Sigmoid, bias=br[:])
                    zp = ps2.tile([128, CH], f32, tag="q")
                    mmq(zp, gks[:, 0:F], A[:, o:o + wc], grw[:, 0:F], xt2)
                    z = p2.tile([128, wc], f16, tag="z")
                    nc.scalar.activation(z[:], zp[:, 0:wc], AF.Sigmoid, bias=bz[:])
                    # gq = gr_h^T xt
                    gq = ps2.tile([128, CH], f32, tag="q")
                    mmq(gq, grw[:, 2 * F:3 * F], xt2)
                    u = p2.tile([128, wc], f16, tag="u")
                    nc.vector.scalar_tensor_tensor(
                        out=u[:], in0=gq[:, 0:wc], scalar=ms[:, 4:5], in1=r[:],
                        op0=ALU.add, op1=ALU.mult)
                    # hp = gk_h^T A + I u  (t2 folded into PSUM)
                    hp = ps2.tile([128, CH], f32, tag="q")
                    mmq(hp, gks[:, 2 * F:3 * F], A[:, o:o + wc],
                        ident[:], u)
                    hc = p2.tile([128, wc], f16, tag="hc")
                    nc.scalar.activation(hc[:], hp[:, 0:wc], AF.Tanh, bias=bh[:])
                    dd = p2.tile([128, wc], f16, tag="dd")
                    nc.vector.tensor_sub(dd[:], xt2[:], hc[:])
                    oo2 = p2.tile([128, wc], f16, tag="oo2")
                    nc.vector.tensor_mul(oo2[:], dd[:], z[:])
                    oo = p2.tile([128, wc], f16, tag="oo")
                    nc.vector.tensor_add(oo[:], oo2[:], hc[:])
                    nc.sync.dma_start(out_d[:, o:o + wc], oo[:])

    return nc


# ---------------------------------------------------------------- kernel

def kernel(**inputs):
    _install_ntff_hook()
    from concourse.bass_utils import run_bass_kernel_spmd
    meta, in_maps, colmaps = _prepare(inputs)
    nc = _build_program(meta)
    _split_excess_waits(nc, cap=1)
    trace = bool(int(os.environ.get("KERNEL_TRACE", "0")))
    kw = {}
    if trace:
        kw = dict(trace=True,
                  tmpdir=os.environ.get("KERNEL_TRACE_DIR",
                                        "/tmp/kernel_trace"))
    res = run_bass_kernel_spmd(nc, in_maps, list(range(NCORE)), **kw)
    if trace:
        kernel.last_exec_time_ns = res.exec_time_ns
    ncn = meta["ncn"]
    outs = []
    for c in range(NCORE):
        op = np.asarray(res.results[c]["outT"], dtype=np.float32)
        on = np.empty_like(op)
        on[:, colmaps[c]] = op
        outs.append(on.T)
    return np.concatenate(outs, axis=0)


kernel.last_exec_time_ns = None
